# revision 24
# baseline (speedup 1.0000x reference)
"""Trainium2 Bass kernel for nn_BulkSpaceGenerator.

Computes, for boundary_tokens x (B, N, D), W1 (D, K*D), b1 (K*D,):
    bulk   = x @ W1 + b1                    -> (B, N, K, D)
    inc    = |delta_n bulk| * (ads/z_k)     (delta along sequence, first row = bulk[0])
    out    = cumsum_n(inc).mean(k)          -> (B, N, D)

Key algebraic restructuring:
  - mean over k commutes with the cumsum, so out = cumsum_n(mean_k(warp_k*|delta|)).
  - delta_n bulk = (delta_n x) @ W1 (bias cancels for n>0), so we matmul the
    *differenced* input once instead of materializing bulk.
  - the warp scale warp_k/K = 1/(k+1) is positive, so it commutes with the abs:
    it is applied (together with the fp8 descale) as the `scale` operand of the
    Abs activation that evacuates each PSUM tile.

The main matmul runs in fp8 (TRN FP8_EXP4) with perf_mode=DoubleRow, packing
two 128-row contraction blocks per instruction (2 fp8 mults/cell/cycle). dx is
quantized x16, W1 x32 (both clipped to +-240); the 1/512 descale folds into the
per-k evacuation scale. The 2e-2 rel-err budget dwarfs the ~2e-3 this costs.

Sharding: 8 shards over (B=2) x (4 sequence chunks of 1024 tokens). Each core
computes its chunk's per-token increments m = sum_k warp_k |dx @ W1|_k and the
local cumsum on-device; the host adds the (tiny) cross-chunk prefix offsets.

Device layout per core (tokens on PSUM partitions, so the local cumsum is a
matmul with a triangular ones matrix and output rows DMA out contiguously):
  dxt  (128, 8, 1024) fp8   [p, cb, t]  = 16*dx_chunk[t, cb*128+p]
  w1s  (128, 8, 10240) fp8  [p, cb, kd] = 32*W1[cb*128+p, kd]
  out  (1024, 1024) bf16    local cumsum of m over the chunk

Cumsum via a persistent PSUM carry tile per d-half: for each 128-token block
tb, C += tri@m[tb] makes C the output rows of block tb (copied out and DMA'd
in bf16), then C += slt@m[tb] (strictly-lower-triangular ones) turns C into
the carry for block tb+1. The fix-up matmul is emitted lagged so the copy has
long completed: no DVE running-sum chain, no hi/lo splits.
"""

import os
import sys
import types
import numpy as np
import ml_dtypes

D = 1024
K = 10
B = 2
N = 4096
ADS_RADIUS = 1.0
NCORES = 8
CHUNK = 1024            # tokens per core
KD = K * D
CB = 8                  # contraction blocks (D / 128)
TBLK = 8                # token blocks per chunk (CHUNK / 128)
DX0T = 3                # token blocks in the first dx transfer

BF16 = ml_dtypes.bfloat16
FP8 = ml_dtypes.float8_e4m3      # TRN FP8_EXP4: max normal 240, inf at 256
FP8_MAX = 240.0
DX_SCALE = 16.0                  # dx ~ N(0, sqrt(2)); x16 keeps 6.5 sigma < 240
W_SCALE = 32.0                   # W1 ~ N(0, 1/32); x32 normalizes to sigma 1

# kd column groups streamed from HBM, in 512-col tiles. One dma_start per
# group (a single InstDMACopy spreads over all 16 SDMA engines; per-issue
# DIRECT2D cost on the sequencer is ~0.6us, so fewer+bigger wins). The first
# group is small so the PE starts within ~10us. These cover pass A (kd tiles
# 0..9, k=0..4), streamed column-major (tokens inner) to match DMA arrival;
# pass B (kd tiles 10..19) runs token-major off a fully resident weight half
# so each block's m completes ~9us apart and the serial cumsum chain hides
# under pass B's matmuls.
GROUP_TILES = [1, 2, 3, 4]
PA_TILES = sum(GROUP_TILES)          # 10 kd tiles in pass A
PB_TILES = KD // 512 - PA_TILES      # 10 kd tiles in pass B

_CACHE = {}


def _install_ntff_hook():
    """Best-effort: register the axon NTFF profiling hook so BASS_TRACE=1 works.

    The agent image's antenv package lacks axon_hooks; inject a shim module and
    wire it to the ctypes-based hook from trn_agent_boot. Harmless if anything
    is missing -- tracing is simply skipped.
    """
    try:
        import antenv
        if "antenv.axon_hooks" in sys.modules:
            return
        hooks = []
        mod = types.ModuleType("antenv.axon_hooks")
        mod.set_axon_ntff_profile_hook = hooks.append
        mod.get_axon_ntff_profile_hook = lambda: (hooks[-1] if hooks else None)
        sys.modules["antenv.axon_hooks"] = mod
        antenv.axon_hooks = mod
        from trn_agent_boot.trn_boot import _ntff_profile_via_ctypes
        h = _ntff_profile_via_ctypes("/opt/axon/libaxon_pjrt.so")
        if h is not None:
            mod.set_axon_ntff_profile_hook(h)
    except Exception:
        pass


def _build():
    from concourse import bacc
    import concourse.mybir as mybir
    import concourse.tile as tile

    fp32 = mybir.dt.float32
    bf16 = mybir.dt.bfloat16
    fp8 = mybir.dt.float8e4
    ADD = mybir.AluOpType.add
    DOUBLE_ROW = mybir.MatmulPerfMode.DoubleRow

    nc = bacc.Bacc()
    dxt = nc.declare_dram_parameter("dxt", [128, CB, CHUNK], fp8, isOutput=False)
    w1s = nc.declare_dram_parameter("w1s", [128, CB, KD], fp8, isOutput=False)
    tri = nc.declare_dram_parameter("tri", [128, 128], bf16, isOutput=False)
    slt = nc.declare_dram_parameter("slt", [128, 128], bf16, isOutput=False)
    # p-major output layout: [p, tb, d] = token tb*128+p; host transposes.
    # Lets block-range DMAs read one contiguous SBUF/HBM run per partition.
    out = nc.declare_dram_parameter("out", [128, TBLK, D], bf16, isOutput=True)

    with tile.TileContext(nc) as tc:
        with (
            tc.tile_pool(name="const", bufs=1) as cpool,
            tc.tile_pool(name="dx", bufs=1) as dxpool,
            tc.tile_pool(name="w", bufs=2) as wpool,
            tc.tile_pool(name="wB", bufs=1) as wbpool,
            tc.tile_pool(name="acc", bufs=1) as accpool,
            tc.tile_pool(name="tmp", bufs=4) as tpool,
            tc.tile_pool(name="psum", bufs=6, space="PSUM") as ppool,
            tc.tile_pool(name="carry", bufs=1, space="PSUM") as carpool,
        ):
            # Each HWDGE ring retires DMAs serially with ~2us fixed completion
            # latency apiece, so ring order is the head's critical path: dx
            # rides the ACT ring FIRST (the first matmul gates on it); the
            # weight stream owns the sync ring; tri/slt slot in behind w1.
            dx0_sb = dxpool.tile([128, CB, DX0T * 128], fp8, tag="dx0")
            dxr_sb = dxpool.tile([128, CB, CHUNK - DX0T * 128], fp8, tag="dxr")
            nc.scalar.dma_start(out=dx0_sb[:], in_=dxt[:, :, 0:DX0T * 128])
            nc.scalar.dma_start(out=dxr_sb[:], in_=dxt[:, :, DX0T * 128:])

            tri_sb = cpool.tile([128, 128], bf16, tag="tri")
            slt_sb = cpool.tile([128, 128], bf16, tag="slt")

            def dx_lhsT(cb, tb):
                if tb < DX0T:
                    return dx0_sb[:, cb:cb + 2, tb * 128:(tb + 1) * 128]
                t = tb - DX0T
                return dxr_sb[:, cb:cb + 2, t * 128:(t + 1) * 128]

            # acc in bf16: DVE accumulate runs in 2x 16-bit mode and the
            # cumsum matmuls read it directly (no separate bf16 snapshot).
            # ~20 bf16 adds cost ~1.4% RMS on m -> ~3e-4 output Frobenius.
            acc = accpool.tile([128, TBLK, D], bf16, tag="acc")
            outbuf = accpool.tile([128, TBLK, D], bf16, tag="outbuf")

            # persistent PSUM carry, one 512-col bank per d-half
            carry = carpool.tile([128, 2, 512], fp32, tag="carry")

            def emit_fix(tb):
                # carry += slt @ m[tb]: converts "output rows of block tb"
                # into the carry-in for block tb+1
                for h in range(2):
                    nc.tensor.matmul(
                        carry[:, h, :], lhsT=slt_sb[:],
                        rhs=acc[:, tb, h * 512:(h + 1) * 512],
                        start=False, stop=False, skip_group_check=True,
                    )

            def emit_out(tb):
                # carry += tri @ m[tb] makes carry == output rows of block tb
                for h in range(2):
                    nc.tensor.matmul(
                        carry[:, h, :], lhsT=tri_sb[:],
                        rhs=acc[:, tb, h * 512:(h + 1) * 512],
                        start=(tb == 0), stop=(tb == TBLK - 1),
                        skip_group_check=True,
                    )
                # copy the two halves on ACT and DVE in parallel
                nc.scalar.copy(outbuf[:, tb, 0:512], carry[:, 0, :])
                nc.vector.tensor_copy(outbuf[:, tb, 512:1024], carry[:, 1, :])

            # grouped output DMAs, alternating rings: (blocks, ring)
            OUT_DMA = {1: (0, 2, "scalar"), 3: (2, 4, "sync"),
                       5: (4, 6, "scalar"), 6: (6, 7, "sync"),
                       7: (7, 8, "scalar")}

            def emit_out_dma(last_done):
                if last_done in OUT_DMA:
                    a, b, ring = OUT_DMA[last_done]
                    eng = nc.scalar if ring == "scalar" else nc.sync
                    eng.dma_start(out=out[:, a:b, :], in_=outbuf[:, a:b, :])

            def evac_one(tb, kd_tile, pstile):
                doff = (kd_tile % 2) * 512            # k = kd_tile // 2
                # warp_k/K = 1/(k+1), plus the fp8 input descale 1/512
                sc = 1.0 / ((kd_tile // 2 + 1) * DX_SCALE * W_SCALE)
                a = acc[:, tb, doff:doff + 512]
                if kd_tile < 2:
                    # first k for this d-half: acc = |sc * psum| (ScalarE)
                    nc.scalar.activation(
                        a, pstile[:], mybir.ActivationFunctionType.Abs,
                        scale=sc,
                    )
                else:
                    # abs on ScalarE (PSUM->SBUF), accumulate on VectorE
                    tmp = tpool.tile([128, 512], bf16, tag="tmp", name="tmp")
                    nc.scalar.activation(
                        tmp[:], pstile[:], mybir.ActivationFunctionType.Abs,
                        scale=sc,
                    )
                    nc.vector.tensor_tensor(a, a, tmp[:], ADD)

            def mains(tb, wtile, wcols_off, jt, kd_base):
                ps = [ppool.tile([128, 512], fp32, tag="ps", name=f"ps{j}") for j in range(jt)]
                for cb in range(0, CB, 2):
                    # fp8 DoubleRow: both operands [128, 2, free] -- two
                    # contraction blocks per instruction, 2 mults/cell/cyc
                    lhsT = dx_lhsT(cb, tb)
                    for j in range(jt):
                        co = wcols_off + j * 512
                        nc.tensor.matmul(
                            ps[j][:],
                            lhsT=lhsT,
                            rhs=wtile[:, cb:cb + 2, co:co + 512],
                            start=(cb == 0),
                            stop=(cb == CB - 2),
                            perf_mode=DOUBLE_ROW,
                        )
                for j in range(jt):
                    evac_one(tb, kd_base + j, ps[j])

            # ---- pass A: kd tiles 0..9, column groups outer, tokens inner
            wB = wbpool.tile([128, CB, PB_TILES * 512], fp8, tag="wB")
            kd_base = 0
            for g, jt in enumerate(GROUP_TILES):
                gcols = jt * 512
                wt = wpool.tile([128, CB, gcols], fp8, tag="wt", name="wt")
                nc.sync.dma_start(
                    out=wt[:], in_=w1s[:, :, kd_base * 512:kd_base * 512 + gcols]
                )
                if g == 2:
                    # pass B's weight half rides behind w2 (its own pool slot,
                    # no WAR on the ring head); w1 must not queue behind its
                    # 5 MB or pass A's g1 starts late. tri/slt follow: first
                    # needed by pass B's cumsum.
                    nc.sync.dma_start(out=wB[:], in_=w1s[:, :, PA_TILES * 512:])
                    nc.sync.dma_start(out=tri_sb[:], in_=tri[:])
                    nc.sync.dma_start(out=slt_sb[:], in_=slt[:])
                for tb in range(TBLK):
                    mains(tb, wt, 0, jt, kd_base)
                kd_base += jt

            # ---- pass B: kd tiles 10..19, tokens outer; each block's m
            # completes ~9us apart, so the serial carry chain (tri -> copies
            # -> slt, ~3.4us/block) hides entirely under the matmuls.
            for tb in range(TBLK):
                mains(tb, wB, 0, PB_TILES, PA_TILES)
                if tb >= 2:
                    emit_fix(tb - 2)
                if tb >= 1:
                    emit_out(tb - 1)
                    emit_out_dma(tb - 1)
            emit_fix(TBLK - 2)
            emit_out(TBLK - 1)
            emit_out_dma(TBLK - 1)

    nc.compile()
    return nc


def _get_nc():
    if "nc" not in _CACHE:
        _CACHE["nc"] = _build()
    return _CACHE["nc"]


def kernel(boundary_tokens: np.ndarray, W1: np.ndarray, b1: np.ndarray) -> np.ndarray:
    from concourse.bass_utils import run_bass_kernel_spmd

    _install_ntff_hook()

    x = np.asarray(boundary_tokens, dtype=np.float32)
    W1 = np.asarray(W1, dtype=np.float32)
    b1 = np.asarray(b1, dtype=np.float32)
    assert x.shape == (B, N, D) and W1.shape == (D, KD)

    # host prep: difference along the sequence; quantize to TRN fp8 (e4m3,
    # max 240). The warp scale 1/(k+1) is NOT folded into W1 here (it would
    # push late-k columns into fp8 subnormals) -- the kernel applies it in the
    # Abs-evacuation scale instead.
    dx = np.empty_like(x)
    dx[:, 0] = x[:, 0]
    dx[:, 1:] = x[:, 1:] - x[:, :-1]

    scale = (1.0 / (np.arange(K, dtype=np.float32) + 1.0))  # warp_k / K = 1/(k+1)
    W1q = np.clip(W1 * W_SCALE, -FP8_MAX, FP8_MAX).astype(FP8)
    w1s_in = np.ascontiguousarray(
        W1q.reshape(CB, 128, KD).transpose(1, 0, 2)
    )

    idx = np.arange(128)
    tri = (idx[:, None] <= idx[None, :]).astype(BF16)   # tri[s,t]=1 iff s<=t
    slt = (idx[:, None] > idx[None, :]).astype(BF16)    # slt[s,t]=1 iff s>t

    chunks_per_b = N // CHUNK
    in_maps = []
    for core in range(NCORES):
        b, c = divmod(core, chunks_per_b)
        dxc = dx[b, c * CHUNK:(c + 1) * CHUNK]          # (CHUNK, D)
        dxq = np.clip(dxc.T * DX_SCALE, -FP8_MAX, FP8_MAX).astype(FP8)
        dxt = np.ascontiguousarray(
            dxq.reshape(CB, 128, CHUNK).transpose(1, 0, 2)
        )
        in_maps.append({"dxt": dxt, "w1s": w1s_in, "tri": tri, "slt": slt})

    res = run_bass_kernel_spmd(
        _get_nc(), in_maps, list(range(NCORES)),
        trace=bool(os.environ.get("BASS_TRACE")),
    )
    _CACHE["last_results"] = res

    out = np.empty((B, N, D), dtype=np.float32)
    for b in range(B):
        offset = np.zeros((D,), dtype=np.float32)
        for c in range(chunks_per_b):
            raw = res.results[b * chunks_per_b + c]["out"]   # [128, TBLK, D]
            core_out = raw.transpose(1, 0, 2).reshape(CHUNK, D).astype(np.float32)
            out[b, c * CHUNK:(c + 1) * CHUNK] = core_out + offset[None, :]
            offset = out[b, (c + 1) * CHUNK - 1].copy()

    if np.any(b1 != 0.0):
        # the kernel ignores b1 (it cancels in all diffs except row 0);
        # swap row 0's increment for the exact fp32 one including b1.
        W1q_f = W1q.astype(np.float32)
        for b in range(B):
            d0_q = np.clip(x[b, 0] * DX_SCALE, -FP8_MAX, FP8_MAX).astype(FP8)
            v_kern = (d0_q.astype(np.float32) @ W1q_f).reshape(K, D)
            m_kern = (np.abs(v_kern) * (scale / (DX_SCALE * W_SCALE))[:, None]).sum(axis=0)
            v_true = x[b, 0] @ W1 + b1
            m_true = (np.abs(v_true.reshape(K, D)) * scale[:, None]).sum(axis=0)
            out[b] += (m_true - m_kern)[None, :]

    return out


# revision 27
# speedup vs baseline: 1.0689x; 1.0689x over previous
"""Trainium2 Bass kernel for nn_BulkSpaceGenerator.

Computes, for boundary_tokens x (B, N, D), W1 (D, K*D), b1 (K*D,):
    bulk   = x @ W1 + b1                    -> (B, N, K, D)
    inc    = |delta_n bulk| * (ads/z_k)     (delta along sequence, first row = bulk[0])
    out    = cumsum_n(inc).mean(k)          -> (B, N, D)

Key algebraic restructuring:
  - mean over k commutes with the cumsum, so out = cumsum_n(mean_k(warp_k*|delta|)).
  - delta_n bulk = (delta_n x) @ W1 (bias cancels for n>0), so we matmul the
    *differenced* input once instead of materializing bulk.
  - the warp scale warp_k/K = 1/(k+1) is positive, so it commutes with the abs:
    it is applied (together with the fp8 descale) as the `scale` operand of the
    Abs activation that evacuates each PSUM tile.

The main matmul runs in fp8 (TRN FP8_EXP4) with perf_mode=DoubleRow, packing
two 128-row contraction blocks per instruction (2 fp8 mults/cell/cycle). dx is
quantized x16, W1 x32 (both clipped to +-240); the 1/512 descale folds into the
per-k evacuation scale. The 2e-2 rel-err budget dwarfs the ~2e-3 this costs.

Sharding: 8 shards over (B=2) x (4 sequence chunks of 1024 tokens). Each core
computes its chunk's per-token increments m = sum_k warp_k |dx @ W1|_k and the
local cumsum on-device; the host adds the (tiny) cross-chunk prefix offsets.

Device layout per core (tokens on PSUM partitions, so the local cumsum is a
matmul with a triangular ones matrix and output rows DMA out contiguously):
  dxt  (128, 8, 1024) fp8   [p, cb, t]  = 16*dx_chunk[t, cb*128+p]
  w1s  (128, 8, 10240) fp8  [p, cb, kd] = 32*W1[cb*128+p, kd]
  out  (1024, 1024) bf16    local cumsum of m over the chunk

Cumsum via a persistent PSUM carry tile per d-half: for each 128-token block
tb, C += tri@m[tb] makes C the output rows of block tb (copied out and DMA'd
in bf16), then C += slt@m[tb] (strictly-lower-triangular ones) turns C into
the carry for block tb+1. The fix-up matmul is emitted lagged so the copy has
long completed: no DVE running-sum chain, no hi/lo splits.
"""

import os
import sys
import types
import numpy as np
import ml_dtypes

D = 1024
K = 10
B = 2
N = 4096
ADS_RADIUS = 1.0
NCORES = 8
CHUNK = 1024            # tokens per core
KD = K * D
CB = 8                  # contraction blocks (D / 128)
TBLK = 8                # token blocks per chunk (CHUNK / 128)
DX0T = 3                # token blocks in the first dx transfer

BF16 = ml_dtypes.bfloat16
FP8 = ml_dtypes.float8_e4m3      # TRN FP8_EXP4: max normal 240, inf at 256
FP8_MAX = 240.0
DX_SCALE = 16.0                  # dx ~ N(0, sqrt(2)); x16 keeps 6.5 sigma < 240
W_SCALE = 32.0                   # W1 ~ N(0, 1/32); x32 normalizes to sigma 1

# kd column groups streamed from HBM, in 512-col tiles. One dma_start per
# group (a single InstDMACopy spreads over all 16 SDMA engines; per-issue
# DIRECT2D cost on the sequencer is ~0.6us, so fewer+bigger wins). The first
# group is small so the PE starts within ~10us. These cover pass A (kd tiles
# 0..9, k=0..4), streamed column-major (tokens inner) to match DMA arrival;
# pass B (kd tiles 10..19) runs token-major off a fully resident weight half
# so each block's m completes ~9us apart and the serial cumsum chain hides
# under pass B's matmuls.
GROUP_TILES = [1, 2, 3, 4]
PA_TILES = sum(GROUP_TILES)          # 10 kd tiles in pass A
PB_TILES = KD // 512 - PA_TILES      # 10 kd tiles in pass B

_CACHE = {}


def _install_ntff_hook():
    """Best-effort: register the axon NTFF profiling hook so BASS_TRACE=1 works.

    The agent image's antenv package lacks axon_hooks; inject a shim module and
    wire it to the ctypes-based hook from trn_agent_boot. Harmless if anything
    is missing -- tracing is simply skipped.
    """
    try:
        import antenv
        if "antenv.axon_hooks" in sys.modules:
            return
        hooks = []
        mod = types.ModuleType("antenv.axon_hooks")
        mod.set_axon_ntff_profile_hook = hooks.append
        mod.get_axon_ntff_profile_hook = lambda: (hooks[-1] if hooks else None)
        sys.modules["antenv.axon_hooks"] = mod
        antenv.axon_hooks = mod
        from trn_agent_boot.trn_boot import _ntff_profile_via_ctypes
        h = _ntff_profile_via_ctypes("/opt/axon/libaxon_pjrt.so")
        if h is not None:
            mod.set_axon_ntff_profile_hook(h)
    except Exception:
        pass


def _build():
    from concourse import bacc
    import concourse.mybir as mybir
    import concourse.tile as tile

    fp32 = mybir.dt.float32
    bf16 = mybir.dt.bfloat16
    fp8 = mybir.dt.float8e4
    ADD = mybir.AluOpType.add
    DOUBLE_ROW = mybir.MatmulPerfMode.DoubleRow

    nc = bacc.Bacc()
    dxt = nc.declare_dram_parameter("dxt", [128, CB, CHUNK], fp8, isOutput=False)
    w1s = nc.declare_dram_parameter("w1s", [128, CB, KD], fp8, isOutput=False)
    tri = nc.declare_dram_parameter("tri", [128, 128], bf16, isOutput=False)
    slt = nc.declare_dram_parameter("slt", [128, 128], bf16, isOutput=False)
    # p-major output layout: [p, tb, d] = token tb*128+p; host transposes.
    # Lets block-range DMAs read one contiguous SBUF/HBM run per partition.
    out = nc.declare_dram_parameter("out", [128, TBLK, D], bf16, isOutput=True)

    with tile.TileContext(nc) as tc:
        with (
            tc.tile_pool(name="const", bufs=1) as cpool,
            tc.tile_pool(name="dx", bufs=1) as dxpool,
            tc.tile_pool(name="w", bufs=2) as wpool,
            tc.tile_pool(name="wB", bufs=1) as wbpool,
            tc.tile_pool(name="acc", bufs=1) as accpool,
            tc.tile_pool(name="tmp", bufs=4) as tpool,
            tc.tile_pool(name="psum", bufs=6, space="PSUM") as ppool,
            tc.tile_pool(name="carry", bufs=1, space="PSUM") as carpool,
        ):
            # Each HWDGE ring retires DMAs serially with ~2us fixed completion
            # latency apiece, so ring order is the head's critical path: dx
            # rides the ACT ring FIRST (the first matmul gates on it); the
            # weight stream owns the sync ring; tri/slt slot in behind w1.
            dx0_sb = dxpool.tile([128, CB, DX0T * 128], fp8, tag="dx0")
            dxr_sb = dxpool.tile([128, CB, CHUNK - DX0T * 128], fp8, tag="dxr")
            nc.scalar.dma_start(out=dx0_sb[:], in_=dxt[:, :, 0:DX0T * 128])
            nc.scalar.dma_start(out=dxr_sb[:], in_=dxt[:, :, DX0T * 128:])

            tri_sb = cpool.tile([128, 128], bf16, tag="tri")
            slt_sb = cpool.tile([128, 128], bf16, tag="slt")

            def dx_lhsT(cb, tb):
                if tb < DX0T:
                    return dx0_sb[:, cb:cb + 2, tb * 128:(tb + 1) * 128]
                t = tb - DX0T
                return dxr_sb[:, cb:cb + 2, t * 128:(t + 1) * 128]

            # acc in bf16: DVE accumulate runs in 2x 16-bit mode and the
            # cumsum matmuls read it directly (no separate bf16 snapshot).
            # ~20 bf16 adds cost ~1.4% RMS on m -> ~3e-4 output Frobenius.
            acc = accpool.tile([128, TBLK, D], bf16, tag="acc")
            outbuf = accpool.tile([128, TBLK, D], bf16, tag="outbuf")

            # persistent PSUM carry, one 512-col bank per d-half
            carry = carpool.tile([128, 2, 512], fp32, tag="carry")

            def emit_fix(tb):
                # carry += slt @ m[tb]: converts "output rows of block tb"
                # into the carry-in for block tb+1
                for h in range(2):
                    nc.tensor.matmul(
                        carry[:, h, :], lhsT=slt_sb[:],
                        rhs=acc[:, tb, h * 512:(h + 1) * 512],
                        start=False, stop=False, skip_group_check=True,
                    )

            def emit_out(tb):
                # carry += tri @ m[tb] makes carry == output rows of block tb
                for h in range(2):
                    nc.tensor.matmul(
                        carry[:, h, :], lhsT=tri_sb[:],
                        rhs=acc[:, tb, h * 512:(h + 1) * 512],
                        start=(tb == 0), stop=(tb == TBLK - 1),
                        skip_group_check=True,
                    )
                # copy the two halves on ACT and DVE in parallel
                nc.scalar.copy(outbuf[:, tb, 0:512], carry[:, 0, :])
                nc.vector.tensor_copy(outbuf[:, tb, 512:1024], carry[:, 1, :])

            # grouped output DMAs, alternating rings: (blocks, ring)
            OUT_DMA = {1: (0, 2, "scalar"), 3: (2, 4, "sync"),
                       5: (4, 6, "scalar"), 6: (6, 7, "sync")}

            def emit_out_dma(last_done):
                if last_done in OUT_DMA:
                    a, b, ring = OUT_DMA[last_done]
                    eng = nc.scalar if ring == "scalar" else nc.sync
                    eng.dma_start(out=out[:, a:b, :], in_=outbuf[:, a:b, :])
                elif last_done == TBLK - 1:
                    # final block: halves on both rings in parallel (tail path)
                    tb = TBLK - 1
                    nc.scalar.dma_start(out=out[:, tb, 0:512], in_=outbuf[:, tb, 0:512])
                    nc.sync.dma_start(out=out[:, tb, 512:1024], in_=outbuf[:, tb, 512:1024])

            def evac_one(tb, kd_tile, pstile):
                doff = (kd_tile % 2) * 512            # k = kd_tile // 2
                # warp_k/K = 1/(k+1), plus the fp8 input descale 1/512
                sc = 1.0 / ((kd_tile // 2 + 1) * DX_SCALE * W_SCALE)
                a = acc[:, tb, doff:doff + 512]
                if kd_tile < 2:
                    # first k for this d-half: acc = |sc * psum| (ScalarE)
                    nc.scalar.activation(
                        a, pstile[:], mybir.ActivationFunctionType.Abs,
                        scale=sc,
                    )
                else:
                    # abs on ScalarE (PSUM->SBUF), accumulate on VectorE
                    tmp = tpool.tile([128, 512], bf16, tag="tmp", name="tmp")
                    nc.scalar.activation(
                        tmp[:], pstile[:], mybir.ActivationFunctionType.Abs,
                        scale=sc,
                    )
                    nc.vector.tensor_tensor(a, a, tmp[:], ADD)

            def mains(tb, wtile, wcols_off, jt, kd_base):
                ps = [ppool.tile([128, 512], fp32, tag="ps", name=f"ps{j}") for j in range(jt)]
                for cb in range(0, CB, 2):
                    # fp8 DoubleRow: both operands [128, 2, free] -- two
                    # contraction blocks per instruction, 2 mults/cell/cyc
                    lhsT = dx_lhsT(cb, tb)
                    for j in range(jt):
                        co = wcols_off + j * 512
                        nc.tensor.matmul(
                            ps[j][:],
                            lhsT=lhsT,
                            rhs=wtile[:, cb:cb + 2, co:co + 512],
                            start=(cb == 0),
                            stop=(cb == CB - 2),
                            perf_mode=DOUBLE_ROW,
                        )
                for j in range(jt):
                    evac_one(tb, kd_base + j, ps[j])

            # ---- pass A: kd tiles 0..9, column groups outer, tokens inner
            wB = wbpool.tile([128, CB, PB_TILES * 512], fp8, tag="wB")
            kd_base = 0
            for g, jt in enumerate(GROUP_TILES):
                gcols = jt * 512
                wt = wpool.tile([128, CB, gcols], fp8, tag="wt", name="wt")
                nc.sync.dma_start(
                    out=wt[:], in_=w1s[:, :, kd_base * 512:kd_base * 512 + gcols]
                )
                if g == 2:
                    # tri/slt: tiny, first needed by pass B's cumsum
                    nc.sync.dma_start(out=tri_sb[:], in_=tri[:])
                    nc.sync.dma_start(out=slt_sb[:], in_=slt[:])
                if g == 3:
                    # pass B's weight half goes LAST: DMA sems fire roughly in
                    # proportion to total bytes in flight, so queueing these
                    # 5 MB any earlier delays every pass-A group's gate
                    nc.sync.dma_start(out=wB[:], in_=w1s[:, :, PA_TILES * 512:])
                for tb in range(TBLK):
                    mains(tb, wt, 0, jt, kd_base)
                kd_base += jt

            # ---- pass B: kd tiles 10..19, tokens outer; each block's m
            # completes ~9us apart, so the serial carry chain (tri -> copies
            # -> slt, ~3.4us/block) hides entirely under the matmuls.
            for tb in range(TBLK):
                mains(tb, wB, 0, PB_TILES, PA_TILES)
                if tb >= 2:
                    emit_fix(tb - 2)
                if tb >= 1:
                    emit_out(tb - 1)
                    emit_out_dma(tb - 1)
            emit_fix(TBLK - 2)
            emit_out(TBLK - 1)
            emit_out_dma(TBLK - 1)

    nc.compile()
    return nc


def _get_nc():
    if "nc" not in _CACHE:
        _CACHE["nc"] = _build()
    return _CACHE["nc"]


def kernel(boundary_tokens: np.ndarray, W1: np.ndarray, b1: np.ndarray) -> np.ndarray:
    from concourse.bass_utils import run_bass_kernel_spmd

    _install_ntff_hook()

    x = np.asarray(boundary_tokens, dtype=np.float32)
    W1 = np.asarray(W1, dtype=np.float32)
    b1 = np.asarray(b1, dtype=np.float32)
    assert x.shape == (B, N, D) and W1.shape == (D, KD)

    # host prep: difference along the sequence; quantize to TRN fp8 (e4m3,
    # max 240). The warp scale 1/(k+1) is NOT folded into W1 here (it would
    # push late-k columns into fp8 subnormals) -- the kernel applies it in the
    # Abs-evacuation scale instead.
    dx = np.empty_like(x)
    dx[:, 0] = x[:, 0]
    dx[:, 1:] = x[:, 1:] - x[:, :-1]

    scale = (1.0 / (np.arange(K, dtype=np.float32) + 1.0))  # warp_k / K = 1/(k+1)
    W1q = np.clip(W1 * W_SCALE, -FP8_MAX, FP8_MAX).astype(FP8)
    w1s_in = np.ascontiguousarray(
        W1q.reshape(CB, 128, KD).transpose(1, 0, 2)
    )

    idx = np.arange(128)
    tri = (idx[:, None] <= idx[None, :]).astype(BF16)   # tri[s,t]=1 iff s<=t
    slt = (idx[:, None] > idx[None, :]).astype(BF16)    # slt[s,t]=1 iff s>t

    chunks_per_b = N // CHUNK
    in_maps = []
    for core in range(NCORES):
        b, c = divmod(core, chunks_per_b)
        dxc = dx[b, c * CHUNK:(c + 1) * CHUNK]          # (CHUNK, D)
        dxq = np.clip(dxc.T * DX_SCALE, -FP8_MAX, FP8_MAX).astype(FP8)
        dxt = np.ascontiguousarray(
            dxq.reshape(CB, 128, CHUNK).transpose(1, 0, 2)
        )
        in_maps.append({"dxt": dxt, "w1s": w1s_in, "tri": tri, "slt": slt})

    res = run_bass_kernel_spmd(
        _get_nc(), in_maps, list(range(NCORES)),
        trace=bool(os.environ.get("BASS_TRACE")),
    )
    _CACHE["last_results"] = res

    out = np.empty((B, N, D), dtype=np.float32)
    for b in range(B):
        offset = np.zeros((D,), dtype=np.float32)
        for c in range(chunks_per_b):
            raw = res.results[b * chunks_per_b + c]["out"]   # [128, TBLK, D]
            core_out = raw.transpose(1, 0, 2).reshape(CHUNK, D).astype(np.float32)
            out[b, c * CHUNK:(c + 1) * CHUNK] = core_out + offset[None, :]
            offset = out[b, (c + 1) * CHUNK - 1].copy()

    if np.any(b1 != 0.0):
        # the kernel ignores b1 (it cancels in all diffs except row 0);
        # swap row 0's increment for the exact fp32 one including b1.
        W1q_f = W1q.astype(np.float32)
        for b in range(B):
            d0_q = np.clip(x[b, 0] * DX_SCALE, -FP8_MAX, FP8_MAX).astype(FP8)
            v_kern = (d0_q.astype(np.float32) @ W1q_f).reshape(K, D)
            m_kern = (np.abs(v_kern) * (scale / (DX_SCALE * W_SCALE))[:, None]).sum(axis=0)
            v_true = x[b, 0] @ W1 + b1
            m_true = (np.abs(v_true.reshape(K, D)) * scale[:, None]).sum(axis=0)
            out[b] += (m_true - m_kern)[None, :]

    return out


# revision 28
# speedup vs baseline: 1.0804x; 1.0108x over previous
"""Trainium2 Bass kernel for nn_BulkSpaceGenerator.

Computes, for boundary_tokens x (B, N, D), W1 (D, K*D), b1 (K*D,):
    bulk   = x @ W1 + b1                    -> (B, N, K, D)
    inc    = |delta_n bulk| * (ads/z_k)     (delta along sequence, first row = bulk[0])
    out    = cumsum_n(inc).mean(k)          -> (B, N, D)

Key algebraic restructuring:
  - mean over k commutes with the cumsum, so out = cumsum_n(mean_k(warp_k*|delta|)).
  - delta_n bulk = (delta_n x) @ W1 (bias cancels for n>0), so we matmul the
    *differenced* input once instead of materializing bulk.
  - the warp scale warp_k/K = 1/(k+1) is positive, so it commutes with the abs:
    it is applied (together with the fp8 descale) as the `scale` operand of the
    Abs activation that evacuates each PSUM tile.

The main matmul runs in fp8 (TRN FP8_EXP4) with perf_mode=DoubleRow, packing
two 128-row contraction blocks per instruction (2 fp8 mults/cell/cycle). dx is
quantized x16, W1 x32 (both clipped to +-240); the 1/512 descale folds into the
per-k evacuation scale. The 2e-2 rel-err budget dwarfs the ~2e-3 this costs.

Sharding: 8 shards over (B=2) x (4 sequence chunks of 1024 tokens). Each core
computes its chunk's per-token increments m = sum_k warp_k |dx @ W1|_k and the
local cumsum on-device; the host adds the (tiny) cross-chunk prefix offsets.

Device layout per core (tokens on PSUM partitions, so the local cumsum is a
matmul with a triangular ones matrix and output rows DMA out contiguously):
  dxt  (128, 8, 1024) fp8   [p, cb, t]  = 16*dx_chunk[t, cb*128+p]
  w1s  (128, 8, 10240) fp8  [p, cb, kd] = 32*W1[cb*128+p, kd]
  out  (1024, 1024) bf16    local cumsum of m over the chunk

Cumsum via a persistent PSUM carry tile per d-half: for each 128-token block
tb, C += tri@m[tb] makes C the output rows of block tb (copied out and DMA'd
in bf16), then C += slt@m[tb] (strictly-lower-triangular ones) turns C into
the carry for block tb+1. The fix-up matmul is emitted lagged so the copy has
long completed: no DVE running-sum chain, no hi/lo splits.
"""

import os
import sys
import types
import numpy as np
import ml_dtypes

D = 1024
K = 10
B = 2
N = 4096
ADS_RADIUS = 1.0
NCORES = 8
CHUNK = 1024            # tokens per core
KD = K * D
CB = 8                  # contraction blocks (D / 128)
TBLK = 8                # token blocks per chunk (CHUNK / 128)
DX0T = 3                # token blocks in the first dx transfer

BF16 = ml_dtypes.bfloat16
FP8 = ml_dtypes.float8_e4m3      # TRN FP8_EXP4: max normal 240, inf at 256
FP8_MAX = 240.0
DX_SCALE = 16.0                  # dx ~ N(0, sqrt(2)); x16 keeps 6.5 sigma < 240
W_SCALE = 32.0                   # W1 ~ N(0, 1/32); x32 normalizes to sigma 1

# kd column groups streamed from HBM, in 512-col tiles. One dma_start per
# group (a single InstDMACopy spreads over all 16 SDMA engines; per-issue
# DIRECT2D cost on the sequencer is ~0.6us, so fewer+bigger wins). The first
# group is small so the PE starts within ~10us. These cover pass A (kd tiles
# 0..9, k=0..4), streamed column-major (tokens inner) to match DMA arrival;
# pass B (kd tiles 10..19) runs token-major off a fully resident weight half
# so each block's m completes ~9us apart and the serial cumsum chain hides
# under pass B's matmuls.
GROUP_TILES = [1, 2, 3, 4]
PA_TILES = sum(GROUP_TILES)          # 10 kd tiles in pass A
PB_TILES = KD // 512 - PA_TILES      # 10 kd tiles in pass B

_CACHE = {}


def _install_ntff_hook():
    """Best-effort: register the axon NTFF profiling hook so BASS_TRACE=1 works.

    The agent image's antenv package lacks axon_hooks; inject a shim module and
    wire it to the ctypes-based hook from trn_agent_boot. Harmless if anything
    is missing -- tracing is simply skipped.
    """
    try:
        import antenv
        if "antenv.axon_hooks" in sys.modules:
            return
        hooks = []
        mod = types.ModuleType("antenv.axon_hooks")
        mod.set_axon_ntff_profile_hook = hooks.append
        mod.get_axon_ntff_profile_hook = lambda: (hooks[-1] if hooks else None)
        sys.modules["antenv.axon_hooks"] = mod
        antenv.axon_hooks = mod
        from trn_agent_boot.trn_boot import _ntff_profile_via_ctypes
        h = _ntff_profile_via_ctypes("/opt/axon/libaxon_pjrt.so")
        if h is not None:
            mod.set_axon_ntff_profile_hook(h)
    except Exception:
        pass


def _build():
    from concourse import bacc
    import concourse.mybir as mybir
    import concourse.tile as tile

    fp32 = mybir.dt.float32
    bf16 = mybir.dt.bfloat16
    fp8 = mybir.dt.float8e4
    ADD = mybir.AluOpType.add
    DOUBLE_ROW = mybir.MatmulPerfMode.DoubleRow

    nc = bacc.Bacc()
    dxt = nc.declare_dram_parameter("dxt", [128, CB, CHUNK], fp8, isOutput=False)
    w1s = nc.declare_dram_parameter("w1s", [128, CB, KD], fp8, isOutput=False)
    tri = nc.declare_dram_parameter("tri", [128, 128], bf16, isOutput=False)
    slt = nc.declare_dram_parameter("slt", [128, 128], bf16, isOutput=False)
    # p-major output layout: [p, tb, d] = token tb*128+p; host transposes.
    # Lets block-range DMAs read one contiguous SBUF/HBM run per partition.
    out = nc.declare_dram_parameter("out", [128, TBLK, D], bf16, isOutput=True)

    with tile.TileContext(nc) as tc:
        with (
            tc.tile_pool(name="const", bufs=1) as cpool,
            tc.tile_pool(name="dx", bufs=1) as dxpool,
            tc.tile_pool(name="w", bufs=2) as wpool,
            tc.tile_pool(name="wB", bufs=1) as wbpool,
            tc.tile_pool(name="acc", bufs=1) as accpool,
            tc.tile_pool(name="tmp", bufs=4) as tpool,
            tc.tile_pool(name="psum", bufs=6, space="PSUM") as ppool,
            tc.tile_pool(name="carry", bufs=1, space="PSUM") as carpool,
        ):
            # Each HWDGE ring retires DMAs serially with ~2us fixed completion
            # latency apiece, so ring order is the head's critical path: dx
            # rides the ACT ring FIRST (the first matmul gates on it); the
            # weight stream owns the sync ring; tri/slt slot in behind w1.
            dx0_sb = dxpool.tile([128, CB, DX0T * 128], fp8, tag="dx0")
            dxr_sb = dxpool.tile([128, CB, CHUNK - DX0T * 128], fp8, tag="dxr")
            nc.scalar.dma_start(out=dx0_sb[:], in_=dxt[:, :, 0:DX0T * 128])
            nc.scalar.dma_start(out=dxr_sb[:], in_=dxt[:, :, DX0T * 128:])

            tri_sb = cpool.tile([128, 128], bf16, tag="tri")
            slt_sb = cpool.tile([128, 128], bf16, tag="slt")

            def dx_lhsT(cb, tb):
                if tb < DX0T:
                    return dx0_sb[:, cb:cb + 2, tb * 128:(tb + 1) * 128]
                t = tb - DX0T
                return dxr_sb[:, cb:cb + 2, t * 128:(t + 1) * 128]

            # acc in bf16: DVE accumulate runs in 2x 16-bit mode and the
            # cumsum matmuls read it directly (no separate bf16 snapshot).
            # ~20 bf16 adds cost ~1.4% RMS on m -> ~3e-4 output Frobenius.
            acc = accpool.tile([128, TBLK, D], bf16, tag="acc")
            outbuf = accpool.tile([128, TBLK, D], bf16, tag="outbuf")

            # persistent PSUM carry, one 512-col bank per d-half
            carry = carpool.tile([128, 2, 512], fp32, tag="carry")

            def emit_fix(tb):
                # carry += slt @ m[tb]: converts "output rows of block tb"
                # into the carry-in for block tb+1
                for h in range(2):
                    nc.tensor.matmul(
                        carry[:, h, :], lhsT=slt_sb[:],
                        rhs=acc[:, tb, h * 512:(h + 1) * 512],
                        start=False, stop=False, skip_group_check=True,
                    )

            def emit_out(tb):
                # carry += tri @ m[tb] makes carry == output rows of block tb
                for h in range(2):
                    nc.tensor.matmul(
                        carry[:, h, :], lhsT=tri_sb[:],
                        rhs=acc[:, tb, h * 512:(h + 1) * 512],
                        start=(tb == 0), stop=(tb == TBLK - 1),
                        skip_group_check=True,
                    )
                # copy the two halves on ACT and DVE in parallel
                nc.scalar.copy(outbuf[:, tb, 0:512], carry[:, 0, :])
                nc.vector.tensor_copy(outbuf[:, tb, 512:1024], carry[:, 1, :])

            # grouped output DMAs, alternating rings: (blocks, ring)
            OUT_DMA = {1: (0, 2, "scalar"), 3: (2, 4, "sync"),
                       5: (4, 6, "scalar"), 6: (6, 7, "sync"),
                       7: (7, 8, "scalar")}

            def emit_out_dma(last_done):
                if last_done in OUT_DMA:
                    a, b, ring = OUT_DMA[last_done]
                    eng = nc.scalar if ring == "scalar" else nc.sync
                    eng.dma_start(out=out[:, a:b, :], in_=outbuf[:, a:b, :])

            def evac_one(tb, kd_tile, pstile):
                doff = (kd_tile % 2) * 512            # k = kd_tile // 2
                # warp_k/K = 1/(k+1), plus the fp8 input descale 1/512
                sc = 1.0 / ((kd_tile // 2 + 1) * DX_SCALE * W_SCALE)
                a = acc[:, tb, doff:doff + 512]
                if kd_tile < 2:
                    # first k for this d-half: acc = |sc * psum| (ScalarE)
                    nc.scalar.activation(
                        a, pstile[:], mybir.ActivationFunctionType.Abs,
                        scale=sc,
                    )
                else:
                    # abs on ScalarE (PSUM->SBUF), accumulate on VectorE
                    tmp = tpool.tile([128, 512], bf16, tag="tmp", name="tmp")
                    nc.scalar.activation(
                        tmp[:], pstile[:], mybir.ActivationFunctionType.Abs,
                        scale=sc,
                    )
                    nc.vector.tensor_tensor(a, a, tmp[:], ADD)

            def mains(tb, wtile, wcols_off, jt, kd_base):
                ps = [ppool.tile([128, 512], fp32, tag="ps", name=f"ps{j}") for j in range(jt)]
                for cb in range(0, CB, 2):
                    # fp8 DoubleRow: both operands [128, 2, free] -- two
                    # contraction blocks per instruction, 2 mults/cell/cyc
                    lhsT = dx_lhsT(cb, tb)
                    for j in range(jt):
                        co = wcols_off + j * 512
                        nc.tensor.matmul(
                            ps[j][:],
                            lhsT=lhsT,
                            rhs=wtile[:, cb:cb + 2, co:co + 512],
                            start=(cb == 0),
                            stop=(cb == CB - 2),
                            perf_mode=DOUBLE_ROW,
                        )
                for j in range(jt):
                    evac_one(tb, kd_base + j, ps[j])

            # ---- pass A: kd tiles 0..9, column groups outer, tokens inner
            wB = wbpool.tile([128, CB, PB_TILES * 512], fp8, tag="wB")
            kd_base = 0
            for g, jt in enumerate(GROUP_TILES):
                gcols = jt * 512
                wt = wpool.tile([128, CB, gcols], fp8, tag="wt", name="wt")
                nc.sync.dma_start(
                    out=wt[:], in_=w1s[:, :, kd_base * 512:kd_base * 512 + gcols]
                )
                if g == 2:
                    # tri/slt: tiny, first needed by pass B's cumsum
                    nc.sync.dma_start(out=tri_sb[:], in_=tri[:])
                    nc.sync.dma_start(out=slt_sb[:], in_=slt[:])
                if g == 3:
                    # pass B's weight half goes LAST: DMA sems fire roughly in
                    # proportion to total bytes in flight, so queueing these
                    # 5 MB any earlier delays every pass-A group's gate
                    nc.sync.dma_start(out=wB[:], in_=w1s[:, :, PA_TILES * 512:])
                for tb in range(TBLK):
                    mains(tb, wt, 0, jt, kd_base)
                kd_base += jt

            # ---- pass B: kd tiles 10..19, tokens outer; each block's m
            # completes ~9us apart, so the serial carry chain (tri -> copies
            # -> slt, ~3.4us/block) hides entirely under the matmuls.
            for tb in range(TBLK):
                mains(tb, wB, 0, PB_TILES, PA_TILES)
                if tb >= 2:
                    emit_fix(tb - 2)
                if tb >= 1:
                    emit_out(tb - 1)
                    emit_out_dma(tb - 1)
            emit_fix(TBLK - 2)
            emit_out(TBLK - 1)
            emit_out_dma(TBLK - 1)

    nc.compile()
    return nc


def _get_nc():
    if "nc" not in _CACHE:
        _CACHE["nc"] = _build()
    return _CACHE["nc"]


def kernel(boundary_tokens: np.ndarray, W1: np.ndarray, b1: np.ndarray) -> np.ndarray:
    from concourse.bass_utils import run_bass_kernel_spmd

    _install_ntff_hook()

    x = np.asarray(boundary_tokens, dtype=np.float32)
    W1 = np.asarray(W1, dtype=np.float32)
    b1 = np.asarray(b1, dtype=np.float32)
    assert x.shape == (B, N, D) and W1.shape == (D, KD)

    # host prep: difference along the sequence; quantize to TRN fp8 (e4m3,
    # max 240). The warp scale 1/(k+1) is NOT folded into W1 here (it would
    # push late-k columns into fp8 subnormals) -- the kernel applies it in the
    # Abs-evacuation scale instead.
    dx = np.empty_like(x)
    dx[:, 0] = x[:, 0]
    dx[:, 1:] = x[:, 1:] - x[:, :-1]

    scale = (1.0 / (np.arange(K, dtype=np.float32) + 1.0))  # warp_k / K = 1/(k+1)
    W1q = np.clip(W1 * W_SCALE, -FP8_MAX, FP8_MAX).astype(FP8)
    w1s_in = np.ascontiguousarray(
        W1q.reshape(CB, 128, KD).transpose(1, 0, 2)
    )

    idx = np.arange(128)
    tri = (idx[:, None] <= idx[None, :]).astype(BF16)   # tri[s,t]=1 iff s<=t
    slt = (idx[:, None] > idx[None, :]).astype(BF16)    # slt[s,t]=1 iff s>t

    chunks_per_b = N // CHUNK
    in_maps = []
    for core in range(NCORES):
        b, c = divmod(core, chunks_per_b)
        dxc = dx[b, c * CHUNK:(c + 1) * CHUNK]          # (CHUNK, D)
        dxq = np.clip(dxc.T * DX_SCALE, -FP8_MAX, FP8_MAX).astype(FP8)
        dxt = np.ascontiguousarray(
            dxq.reshape(CB, 128, CHUNK).transpose(1, 0, 2)
        )
        in_maps.append({"dxt": dxt, "w1s": w1s_in, "tri": tri, "slt": slt})

    res = run_bass_kernel_spmd(
        _get_nc(), in_maps, list(range(NCORES)),
        trace=bool(os.environ.get("BASS_TRACE")),
    )
    _CACHE["last_results"] = res

    out = np.empty((B, N, D), dtype=np.float32)
    for b in range(B):
        offset = np.zeros((D,), dtype=np.float32)
        for c in range(chunks_per_b):
            raw = res.results[b * chunks_per_b + c]["out"]   # [128, TBLK, D]
            core_out = raw.transpose(1, 0, 2).reshape(CHUNK, D).astype(np.float32)
            out[b, c * CHUNK:(c + 1) * CHUNK] = core_out + offset[None, :]
            offset = out[b, (c + 1) * CHUNK - 1].copy()

    if np.any(b1 != 0.0):
        # the kernel ignores b1 (it cancels in all diffs except row 0);
        # swap row 0's increment for the exact fp32 one including b1.
        W1q_f = W1q.astype(np.float32)
        for b in range(B):
            d0_q = np.clip(x[b, 0] * DX_SCALE, -FP8_MAX, FP8_MAX).astype(FP8)
            v_kern = (d0_q.astype(np.float32) @ W1q_f).reshape(K, D)
            m_kern = (np.abs(v_kern) * (scale / (DX_SCALE * W_SCALE))[:, None]).sum(axis=0)
            v_true = x[b, 0] @ W1 + b1
            m_true = (np.abs(v_true.reshape(K, D)) * scale[:, None]).sum(axis=0)
            out[b] += (m_true - m_kern)[None, :]

    return out
